# revision 41
# baseline (speedup 1.0000x reference)
"""Trainium2 Bass kernel for nn_BSplineActivation.

out[n, f] = sum_j basis_j(x[n, f]) * coeffs[f, j] * scaler[f]
with a cubic B-spline basis on a uniform shared knot grid.

Math: with xc = x - center (center = grid midpoint) and h the knot step,
the spline for feature f is evaluated exactly as

    S_f(xc) = SL_f(min(xc, 0)) + SR_f(max(xc, 0)) - S_f(0)
    SL_f(y) = sum_{k=0..4} A_fk  * relu(y - lsh_k)^3    lsh_k = (k-5) h < 0
    SR_f(y) = sum_{k=0..4} Bt_fk * relu(rsh_k - y)^3    rsh_k = (5-k) h > 0

(two-sided truncated-power representation; the min/max clamps make each
side exact on its half and constant on the other, and x outside the
extended grid lands exactly on 0).  Each term is one fused custom DVE op
    acc' = acc + a * relu(min(w,0) - q)^3
so the whole activation is 10 vector-engine instructions per tile.

Layout: features on partitions (per-feature coefficients become
per-partition scalars), tokens on the free dim.  x tiles are transposed
in via the tensor engine (exact movement mode) and transposed back out;
the scalar engine evacuates PSUM.  Data-parallel over 8 NeuronCores on
the flattened token dim.

Performance (per core, 2048x768 shard): ~144us cost-model / ~110-140us
measured via repeat-slope; vector-engine bound at its floor (10 fused
2-source fp32 passes are port-capped at 1 elem/cycle/lane; accumulation
requires 2 sources; any segment decomposition of this 13-dof spline with
vanishing ends needs >= 10 truncated terms).  Input DMA, both transpose
directions, and PSUM evacuation overlap to within ~3us of the DVE-only
ablation.  Uneven ramp/tail chunking (first block (1,3,12) token-tiles,
last (12,3,1)) starts the DVE chain after one transposed tile and
shrinks the output tail.

Engine-offload was explored in depth (tune["carve"]>0 keeps the working
implementation): the K smallest-reach right-side terms are moved off the
DVE as Bt_k*relu2(rsh_k-ym)*relu(rsh_k-ym) with ACT computing
Relu/Square, cubes multiplied on Pool or DVE, per-feature coefficients
applied by fp32r diagonal matmuls accumulating in PSUM on the otherwise
idle PE (1 cycle/row at moving dim >= 256; measured fp32r precision
~2e-4 relative, fine for the 2e-2 tolerance), S0 injected by a rank-1
(-S0)^T x ones matmul, and the PSUM tile consumed directly as the in1
accumulator seed of the DVE chain's first term (DVE reads PSUM, so no
evacuation).  Hard-won constraints: Pool-engine tensor_scalar is a slow
software Q7 path (~10us/2048 cols measured; ym therefore computes on ACT
as Relu(w)), ACT saturates on PSUM-evac + Relu + Square, every exact
decomposition needs >= 3 non-DVE passes per carved term (no relu^3
table function; PSUM accumulate-on-write is PE-only; DMA/Pool cannot
read PSUM; stt MACs are DVE-only), which caps the useful carve at 2.

Current default (w3t, carve=3): the carved terms no longer seed the DVE
chain through a PSUM W3 tile.  Instead each output-transpose PSUM slice
becomes an accumulation group:

    ps2[tok, f] = T(acc_dve) + sum_j (stationary=r3_j-token-tile) @
                                     (moving=diag(Bt_j) block)

i.e. the per-feature-weighted carved cubes land TRANSPOSED directly on
top of the output transpose (ordinary matmuls accumulate over the
is_transpose write; start=True on the transpose, stop=True on the last
diag matmul; skip_group_check needed).  This removes the W3->first-DVE-op
dependency entirely, so the DVE chain (INIT_L + 4 ACC_L + 2 ACC_R = 7
passes, S0 via the INIT op's C3 slot) never waits on the ACT/Pool
K-side, and the -S0 rank-1 seed matmul is gone.  Validated on HW:
rel err 1.268e-2 (fp32r rounding of three carved terms, k=2 with
R^3<=11.4 dominates; tolerance 2e-2), model 121.6us (c2blocks=(5,):
120.9) vs 129.1us for the best carve=2 merge design and 144.5us for the
pure 10-pass DVE floor.  DVE busy 105us, ACT 102us, PE 77us, Pool 68us —
DVE and ACT are now co-saturated; further carve (4+) dies on ACT
(2 passes/term is irreducible: Relu + Square; ln/exp cubes are 3
passes; Pool squares blow the measured-2x-slow Q7 budget).

Measurement: axon-tunnel HW slopes vary +-50% run to run; the
TimelineSim model deltas are the stable signal (prior session validated
model==HW slope).  Numerics are deterministic across runs.

Explored and rejected this session (model numbers): psum_w — PE
transposes write w straight to PSUM and DVE/ACT read it there, killing
the ACT input-evacuation pass (the 8x2KB PSUM caps the w ring at 2-3
chunks and the pipeline serializes: 149-193us; DVE PSUM access is also
+65ns/instr).  evac_in="dma"/Pool (cannot read PSUM).  16-bit DVE
(custom-op rows have no 2x/4x perf modes — InstCustomDveAnt reports
none; TSP fp32 SBUF does run 2x_2p but >=3 standard ops never beat one
fused pass; fp16 cubes fail the error budget anyway).  Multi-term
fused ops (8-stage v3 pipeline; one term needs 7).  Custom ACT pwp
table for the shared cardinal B-spline basis (act tables are a closed
enum keyed to compiled bkt/ctrl binaries).  tokens-on-partitions (PE
contracts over partitions, so per-feature scaling then has no cheap
engine).  tsplit=1 (+21us: coarser pipeline), out-DMA on the ACT hwdge
queue (+11us: ACT SEQ dispatch blocks the saturated ACT engine),
carve_pat interleavings (monotone between the pure configs).
"""

import os
import numpy as np

# The kernel executes through the axon PJRT backend; make sure a
# JAX_PLATFORMS=cpu pin (common for reference-only environments) does not
# hide the NeuronCore devices.  Must run before jax is first imported.
_jp = os.environ.get("JAX_PLATFORMS")
if _jp is not None and "axon" not in _jp:
    os.environ["JAX_PLATFORMS"] = "axon,cpu"

import concourse.bacc as bacc
import concourse.mybir as mybir
import concourse.tile as tile
from concourse import masks
from concourse.dve_spec import (
    Spec, Src0, Src1, C0, C1, C3, Zero, relu, sq, minn, maxx, lower,
    _spill_c3_to_src1, _has_src1,
)
from concourse.dve_ops import DveOp, OPS, _SUB_OPCODE_FOR_NAME, CUSTOM_DVE_SPECS
from concourse.dve_uop import DveOpSpec

N_CORES = 8
P = 128


# --------------------------------------------------------------------------
# custom DVE ops (registered once per process)
# --------------------------------------------------------------------------

def _register(name, spec):
    for op in OPS:
        if op.name == name:
            return op
    row = max(_SUB_OPCODE_FOR_NAME.values()) + 1
    assert row < 0x20, "no free custom-DVE opcode rows"
    _SUB_OPCODE_FOR_NAME[name] = row
    shas = {}
    for ver in ("v3", "v4"):
        try:
            uops = lower(spec, ver=ver)
            shas[ver] = DveOpSpec(
                name=name, opcode=row, uops=uops, rd1_en=_has_src1(spec)
            ).sha(ver)
        except Exception:
            if ver == "v3":
                raise
    op = DveOp(name, spec, subdim=False, uops_sha=shas)
    OPS.append(op)
    CUSTOM_DVE_SPECS[name] = spec
    return op


def _cube(r):
    return sq(r) * r


# acc + a * relu(min(w,0) - q)^3        (left-side truncated power term)
SPLINE_ACC_L = _register(
    "SPLINE_ACC_L_ANT",
    Spec(
        body=Src1 + C0 * _cube(relu(minn(Src0, Zero) - C1)),
        reference=lambda in0, in1, s0, s1, imm2: (
            in1 + s0 * np.maximum(np.minimum(in0.astype(np.float32), 0) - s1, 0) ** 3
        ).astype(np.float32),
    ),
)

# acc + a * relu(q - max(w,0))^3        (right-side term)
SPLINE_ACC_R = _register(
    "SPLINE_ACC_R_ANT",
    Spec(
        body=Src1 + C0 * _cube(relu(C1 - maxx(Src0, Zero))),
        reference=lambda in0, in1, s0, s1, imm2: (
            in1 + s0 * np.maximum(s1 - np.maximum(in0.astype(np.float32), 0), 0) ** 3
        ).astype(np.float32),
    ),
)

# a * relu(min(w,0) - q)^3 + c          (chain seed; c = -S(0) rides C3->Src1)
SPLINE_INIT_L = _register(
    "SPLINE_INIT_L_ANT",
    Spec(
        body=_spill_c3_to_src1(C0 * _cube(relu(minn(Src0, Zero) - C1)) + C3),
        reference=lambda in0, in1, s0, s1, imm2: (
            s0 * np.maximum(np.minimum(in0.astype(np.float32), 0) - s1, 0) ** 3 + in1
        ).astype(np.float32),
    ),
)


# --------------------------------------------------------------------------
# host-side table construction (exact, float64)
# --------------------------------------------------------------------------

def _build_tables(knots, coeffs, scaler):
    knots = np.asarray(knots, np.float64)
    coeffs = np.asarray(coeffs, np.float64)
    scaler = np.asarray(scaler, np.float64)
    F, G = knots.shape
    h = (knots[:, -1] - knots[:, 0]) / (G - 1)
    assert np.allclose(np.diff(knots, axis=1), h[:, None], rtol=0, atol=1e-5), \
        "kernel assumes uniform knots per feature"
    assert np.allclose(h, h[0], rtol=0, atol=1e-9), "kernel assumes shared knot step"
    h = float(h[0])
    center = (knots[:, 0] + knots[:, -1]) / 2
    assert np.allclose(center, center[0], atol=1e-9)
    center = float(center[0])

    c = coeffs * scaler[:, None]                       # (F, nb)
    nb = c.shape[1]
    w4 = np.array([1.0, -4.0, 6.0, -4.0, 1.0]) / 6.0

    dU = np.zeros((F, nb + 4))
    for j in range(nb):
        dU[:, j:j + 5] += c[:, j:j + 1] * w4[None, :]
    A = dU[:, :5] / h ** 3
    lsh = (np.arange(5) - 5.0) * h

    crev = c[:, ::-1]
    dUT = np.zeros((F, nb + 4))
    for j in range(nb):
        dUT[:, j:j + 5] += crev[:, j:j + 1] * w4[None, :]
    Bt = dUT[:, :5] / h ** 3
    rsh = (5.0 - np.arange(5)) * h

    S0 = (A * np.maximum(-lsh, 0.0)[None, :] ** 3).sum(1)
    return A, Bt, lsh, rsh, S0, h, center


NCOL = 12  # 5 left + 5 right + S0neg + pad


def _pack_tab(A, Bt, S0, F):
    fb = F // P
    tab = np.zeros((P, fb * NCOL), np.float32)
    for b in range(fb):
        sl = slice(b * P, (b + 1) * P)
        tab[:, b * NCOL + 0:b * NCOL + 5] = A[sl]
        tab[:, b * NCOL + 5:b * NCOL + 10] = Bt[sl]
        tab[:, b * NCOL + 10] = -S0[sl]
    return tab


def _pack_diag(Bt, rsh, carve, F):
    """Diagonal PE weight matrices diag(Bt_k) for the carved right-side
    terms (k = 5-carve .. 4), per feature block: (P, fb*carve*P)."""
    fb = F // P
    ks = list(range(5 - carve, 5))
    wd = np.zeros((P, fb * carve * P), np.float32)
    idx = np.arange(P)
    for b in range(fb):
        for j, k in enumerate(ks):
            col0 = (b * carve + j) * P
            wd[idx, col0 + idx] = Bt[b * P:(b + 1) * P, k]
    return wd


def _pack_aux(rsh, carve):
    aux = np.zeros((P, 8), np.float32)
    for j, k in enumerate(range(5 - carve, 5)):
        aux[:, j] = np.float32(rsh[k])
    return aux


# --------------------------------------------------------------------------
# bass program
# --------------------------------------------------------------------------

_PROGRAMS = {}


DEFAULT_TUNE = dict(xin=2, w=5, acc=5, outb=2, psin=2, psout=2, tsplit=2,
                    repeat=1, phase="full", evac=4, first_split=2,
                    last_split=2, first_sizes=(1, 2, 5, 8),
                    last_sizes=(8, 5, 2, 1),
                    # w3t: carved terms merge inside the output-transpose
                    # PSUM accumulation group (stationary = r3 token-tile,
                    # moving = diag weight block), so the DVE chain never
                    # waits on the ACT/Pool K-side.
                    w3t=True, lag_out=True, lazy_wd=True, wd_defer=0,
                    dve_head_evac=0, c2blocks=(5,),
                    # psum_w: PE transposes write the feature-major x tile
                    # directly into PSUM and every consumer (DVE chain, ACT
                    # ym) reads it there — the ACT PSUM->SBUF input
                    # evacuation pass disappears entirely.  Requires chunk
                    # sizes <= 8 token-tiles (PSUM bank budget: w 2x2 banks,
                    # psout 2, psk 2).
                    psum_w=False, wbufs=2,
                    # K-side carve (opt-in, default off): move `carve`
                    # smallest-reach right-side truncated-power terms off the
                    # DVE onto ACT (Relu/Square) + PE (fp32r diag-matmul
                    # chains accumulating in PSUM, consumed as the DVE
                    # chain's in1 seed).  Measured on HW: no win over the
                    # pure-DVE floor (Pool engine tensor ops are far slower
                    # than modeled; ACT saturates on evac + relu + square),
                    # so the default stays carve=0.
                    carve=3, kchunk=4, psk=1, rbuf=4, r2buf=8, ymbuf=2,
                    tbuf=3, kmeth=("sp", "sp", "sp"), ym_eng="act")


def build_program(tok, F, lsh, rsh, tune=None):
    """One-core program: xs (tok, F) f32 -> ys (tok, F) f32."""
    tune = {**DEFAULT_TUNE, **(tune or {})}
    key = (tok, F, tuple(lsh), tuple(rsh), tuple(sorted(tune.items())))
    if key in _PROGRAMS:
        return _PROGRAMS[key]

    fb = F // P
    ti = tok // P
    tsplit = tune["tsplit"]
    assert ti % tsplit == 0
    tic = ti // tsplit           # token-tiles per chunk
    ctok = tok // tsplit         # tokens per chunk
    psum_w = tune["psum_w"]
    if psum_w:
        assert tune["phase"] == "full"
        # largest chunk must fit the fixed [P, 8*P] PSUM w tile
        for key_ in ("first_sizes", "last_sizes"):
            if tune.get(key_):
                assert max(tune[key_]) <= 8, (key_, tune[key_])
        assert tic <= 8

    nc = bacc.Bacc("TRN2", target_bir_lowering=False, debug=False,
                   enable_asserts=False)
    xs = nc.dram_tensor("xs", (tok, F), mybir.dt.float32, kind="ExternalInput").ap()
    tabd = nc.dram_tensor("tab", (P, fb * NCOL), mybir.dt.float32,
                          kind="ExternalInput").ap()
    identd = (nc.dram_tensor("ident", (P, P), mybir.dt.float32,
                             kind="ExternalInput").ap()
              if tune.get("dma_ident", False) else None)
    carve = tune["carve"]
    if carve:
        wdd = nc.dram_tensor("wd", (P, fb * carve * P), mybir.dt.float32,
                             kind="ExternalInput").ap()
        auxd = nc.dram_tensor("aux", (P, 8), mybir.dt.float32,
                              kind="ExternalInput").ap()
        s0td = nc.dram_tensor("s0t", (1, fb * P + tune["kchunk"] * P),
                              mybir.dt.float32, kind="ExternalInput").ap()
    ys = nc.dram_tensor("ys", (tok, F), mybir.dt.float32, kind="ExternalOutput").ap()

    xs_v = xs.rearrange("(t p) (b f) -> b p t f", p=P, f=P)
    ys_v = ys.rearrange("(t p) (b f) -> b p t f", p=P, f=P)

    with tile.TileContext(nc) as tc:
        with (
            tc.tile_pool(name="consts", bufs=1) as consts,
            tc.tile_pool(name="xin_pool", bufs=tune["xin"]) as xin_pool,
            tc.tile_pool(name="w_pool", bufs=tune["w"]) as w_pool,
            tc.tile_pool(name="acc_pool", bufs=tune["acc"]) as acc_pool,
            tc.tile_pool(name="out_pool", bufs=tune["outb"]) as out_pool,
            tc.tile_pool(name="psout", bufs=tune["psout"], space="PSUM") as psout_pool,
            tc.tile_pool(name="psk", bufs=tune["psk"], space="PSUM") as psk_pool,
            tc.tile_pool(name="r_pool", bufs=tune["rbuf"]) as r_pool,
            tc.tile_pool(name="r2_pool", bufs=tune["r2buf"]) as r2_pool,
            tc.tile_pool(name="ym_pool", bufs=tune["ymbuf"]) as ym_pool,
            tc.tile_pool(name="t_pool", bufs=tune["tbuf"]) as t_pool,
        ):
            psin_cm = (None if psum_w else
                       tc.tile_pool(name="psin", bufs=tune["psin"], space="PSUM"))
            psin_pool = psin_cm.__enter__() if psin_cm is not None else None
            wps_cm = (tc.tile_pool(name="wps", bufs=tune["wbufs"], space="PSUM")
                      if psum_w else None)
            wps_pool = wps_cm.__enter__() if wps_cm is not None else None
            # first ramp chunk's input DMA ahead of every constant transfer:
            # the first DVE op waits on (DMA -> transpose -> evac) and the
            # SP queue is FIFO, so anything queued before this chunk delays
            # the whole pipeline head
            early_xin = None
            if tune.get("early_x", False) and tune["phase"] != "notrans":
                fs0 = tune.get("first_sizes")
                tic0 = (fs0[0] if fs0 and sum(fs0) == ti
                        else ti // (tune.get("first_split") or tune["tsplit"]))
                early_xin = xin_pool.tile([P, ti, P], mybir.dt.float32)
                nc.sync.dma_start(early_xin[:, 0:tic0, :],
                                  xs_v[0][:, 0:tic0, :])
            cq = (nc.scalar if tune.get("const_q", "sp") == "act"
                  else nc.sync)
            identity = consts.tile([P, P], mybir.dt.float32)
            if tune.get("dma_ident", False):
                cq.dma_start(identity[:], identd[:])
            else:
                masks.make_identity(nc, identity[:])
            tab = consts.tile([P, fb * NCOL], mybir.dt.float32)
            cq.dma_start(tab[:], tabd[:])
            if carve:
                wdr = consts.tile([P, fb * carve * P], mybir.dt.float32r)
                aux = consts.tile([P, 8], mybir.dt.float32)
                nko = fb * P + tune["kchunk"] * P
                s0f = consts.tile([1, nko], mybir.dt.float32)
                s0r = consts.tile([1, nko], mybir.dt.float32r)
                cq.dma_start(aux[:], auxd[:])
                cq.dma_start(s0f[:], s0td[:])
                nc.gpsimd.tensor_copy(s0r[:], s0f[:])
                ones = s0r[:, fb * P:]

            staged_secs = set()

            def stage_sec(wtmp_pool, sec):
                # stage + producer-round one per-block slice of the diag
                # weights to fp32r.  Emitted one block ahead of its consumer
                # so the Pool-engine converts never burst-collide with the
                # per-chunk cube multiplies (a 10us Pool burst here stalled
                # the W3 seed chain for a whole block).
                if sec in staged_secs or sec >= fb * tune["repeat"]:
                    return
                staged_secs.add(sec)
                sec = sec % fb
                c0 = sec * carve * P
                wt = wtmp_pool.tile([P, carve * P], mybir.dt.float32,
                                    tag="wt")
                dq = nc.scalar if tune.get("wd_q", "sp") == "act" else nc.sync
                dq.dma_start(wt[:], wdd[:, c0:c0 + carve * P])
                nc.gpsimd.tensor_copy(wdr[:, c0:c0 + carve * P], wt[:])

            def stage_wdr(wtmp_pool):
                for sec in range(fb):
                    stage_sec(wtmp_pool, sec)

            staged = [False]
            wtmp_pool_cm = tc.tile_pool(name="wtmp", bufs=2)
            wtmp_pool = wtmp_pool_cm.__enter__() if carve else None
            # lag_out: defer each chunk's output stage (PE out-transpose, ACT
            # PSUM evacuation, output DMA) until after the NEXT chunk's
            # K-side has been emitted.  The ACT queue is FIFO per engine, so
            # without this the next chunk's ym/Relu/Square — which gate the
            # next DVE chain via the W3 seed — wait behind the previous
            # chunk's output evacuation.
            pending_out = [None]

            def emit_out(b, cs, tic, cur, outst, dve_evac, r3s=(),
                         ch_carve=0):
                E = tune["evac"]
                fine_out = tune.get("fine_out", True)
                oq = (nc.scalar if (tune.get("out_q", "sp") == "act" or
                                    b in tune.get("out_q_blocks", ()))
                      else nc.sync)
                for t0 in range(0, tic, E):
                    ne = min(E, tic - t0)
                    ps2 = psout_pool.tile([P, E * P], mybir.dt.float32)
                    for e in range(ne):
                        tg0 = t0 + e
                        if ch_carve:
                            # w3t: the carved terms' per-feature-weighted
                            # cubes accumulate into the same PSUM slice the
                            # output transpose writes: ps2 = T(acc_dve)
                            # + sum_j (diag(Bt_j) r3_j)^T.  Stationary =
                            # r3 token-tile, moving = the diag block, so
                            # the product lands token-major directly and
                            # the DVE chain never waits on the K side.
                            nc.tensor.matmul(
                                ps2[:, e * P:(e + 1) * P],
                                cur[:, tg0 * P:(tg0 + 1) * P],
                                identity[:], is_transpose=True,
                                start=True, stop=False,
                                skip_group_check=True)
                            for ji, (j, R3) in enumerate(r3s):
                                cw = (b * carve + j) * P
                                nc.tensor.matmul(
                                    ps2[:, e * P:(e + 1) * P],
                                    R3[:, tg0 * P:(tg0 + 1) * P],
                                    wdr[:, cw:cw + P],
                                    start=False, stop=(ji == len(r3s) - 1),
                                    skip_group_check=True)
                        else:
                            nc.tensor.transpose(ps2[:, e * P:(e + 1) * P],
                                                cur[:, tg0 * P:(tg0 + 1) * P],
                                                identity[:])
                    if dve_evac:
                        # tail chunks evacuate on the DVE, which is idle by
                        # then, so the drain never queues behind ACT work
                        nc.vector.tensor_copy(
                            outst[:, cs + t0:cs + t0 + ne, :],
                            ps2[:, :ne * P])
                    else:
                        nc.scalar.copy(
                            outst[:, cs + t0:cs + t0 + ne, :],
                            ps2[:, :ne * P])
                    if fine_out:
                        oq.dma_start(
                            ys_v[b][:, cs + t0:cs + t0 + ne, :],
                            outst[:, cs + t0:cs + t0 + ne, :])
                if not fine_out:
                    oq.dma_start(ys_v[b][:, cs:cs + tic, :],
                                      outst[:, cs:cs + tic, :])

            def flush_out():
                if pending_out[0] is not None:
                    emit_out(*pending_out[0])
                    pending_out[0] = None
            for b in range(fb * tune["repeat"]):
                b = b % fb
                tsplit = tune["tsplit"]
                sizes = None
                if (b == 0 and tune.get("first_sizes")
                        and sum(tune["first_sizes"]) == ti):
                    sizes = list(tune["first_sizes"])
                elif (b == fb - 1 and tune.get("last_sizes")
                        and sum(tune["last_sizes"]) == ti):
                    sizes = list(tune["last_sizes"])
                elif b == 0 and tune["first_split"]:
                    tsplit = tune["first_split"]
                elif b == fb - 1 and tune["last_split"]:
                    tsplit = tune["last_split"]
                if sizes is None:
                    assert ti % tsplit == 0
                    sizes = [ti // tsplit] * tsplit
                assert sum(sizes) == ti
                starts = [sum(sizes[:i]) for i in range(len(sizes))]
                def col(j, b=b):
                    return tab[:, b * NCOL + j:b * NCOL + j + 1]

                phase = tune["phase"]
                do_trans = phase in ("full", "nodve")
                do_dve = phase in ("full", "notrans")
                cbs0 = tune.get("cblk_skip", (0,))
                if cbs0 is True:
                    cbs0 = (0, fb - 1)
                elif cbs0 is False:
                    cbs0 = ()
                if carve and tune.get("lazy_wd", False):
                    if b != 0 or tune.get("wd_defer", 1) == 0:
                        # block 0's staging is deferred until after its
                        # first chunk's input DMA (emitted below) so the wd
                        # transfers never delay the first transposes
                        if b not in cbs0:
                            stage_sec(wtmp_pool, b)
                        for bn in range(b + 1, b + fb):
                            if bn % fb not in cbs0:
                                stage_sec(wtmp_pool, bn % fb)
                                break
                elif carve and not staged[0] and b not in cbs0:
                    stage_wdr(wtmp_pool)
                    staged[0] = True

                outst = out_pool.tile([P, ti, P], mybir.dt.float32)
                if do_trans:
                    if b == 0 and early_xin is not None:
                        xin = early_xin
                    else:
                        xin = xin_pool.tile([P, ti, P], mybir.dt.float32)
                for c, (cs, tic) in enumerate(zip(starts, sizes)):
                    ctok = tic * P
                    # small ramp chunks and (optionally) the first/last
                    # feature blocks skip the carve (full DVE chain) so the
                    # K-side pipeline never gates the head/tail; block 0 may
                    # use its own larger threshold (cmin0) so only its final
                    # ramp chunk carves, after the K-side pipeline warmed up
                    # behind the earlier full-DVE chunks
                    cmin = (tune.get("cmin0", 4) if b == 0
                            else tune.get("cmin", 4))
                    ch_carve = carve if tic >= cmin else 0
                    cpat = tune.get("carve_pat")
                    if cpat and ch_carve:
                        # cycle the carve depth across chunks: balances the
                        # DVE chain length against the ACT-side K latency
                        ch_carve = min(ch_carve,
                                       cpat[(b * len(sizes) + c) % len(cpat)])
                    if b in tune.get("c2blocks", ()):
                        # blocks where the ACT queue is the limiter run a
                        # shallower carve
                        ch_carve = min(ch_carve, 2)
                    cbs = tune.get("cblk_skip", (0,))
                    if cbs is True:
                        cbs = (0, fb - 1)
                    elif cbs is False:
                        cbs = ()
                    if b in cbs:
                        ch_carve = 0
                    if psum_w:
                        # fixed-size PSUM tile (2 banks); chunk uses a prefix
                        w = wps_pool.tile([P, 8 * P], mybir.dt.float32,
                                          tag="w")
                        nc.sync.dma_start(xin[:, cs:cs + tic, :],
                                          xs_v[b][:, cs:cs + tic, :])
                        for e in range(tic):
                            nc.tensor.transpose(w[:, e * P:(e + 1) * P],
                                                xin[:, cs + e, :],
                                                identity[:])
                    else:
                        w = w_pool.tile([P, ctok], mybir.dt.float32, tag="w")
                    wt = w[:, :ctok]
                    if do_trans and not psum_w:
                        if not (b == 0 and c == 0 and early_xin is not None):
                            nc.sync.dma_start(xin[:, cs:cs + tic, :],
                                              xs_v[b][:, cs:cs + tic, :])
                        E = tune["evac"]
                        for t0 in range(0, tic, E):
                            ne = min(E, tic - t0)
                            ps = psin_pool.tile([P, E * P], mybir.dt.float32)
                            for e in range(ne):
                                tg = cs + t0 + e
                                nc.tensor.transpose(ps[:, e * P:(e + 1) * P],
                                                    xin[:, tg, :], identity[:])
                            if ((b == 0 and
                                    c < tune.get("dve_head_evac", 1)) or
                                    b in tune.get("dve_evac_blocks", ())):
                                # head chunks evacuate on the DVE so the
                                # first spline op never waits on the ACT
                                # table load; late blocks can use the
                                # draining DVE to relieve the ACT backlog
                                nc.vector.tensor_copy(
                                    w[:, t0 * P:(t0 + ne) * P],
                                    ps[:, :ne * P])
                            elif tune.get("evac_in", "act") == "dma":
                                nc.sync.dma_start(w[:, t0 * P:(t0 + ne) * P],
                                                  ps[:, :ne * P])
                            else:
                                nc.scalar.copy(w[:, t0 * P:(t0 + ne) * P],
                                               ps[:, :ne * P])
                    elif not do_trans:
                        wv = w[:].rearrange("p (t f) -> p t f", f=P)
                        nc.sync.dma_start(
                            wv, xs_v[b][:, cs:cs + tic, :])
                    if (carve and tune.get("lazy_wd", False) and b == 0
                            and c == tune.get("wd_defer", 1) - 1
                            and tune.get("wd_defer", 1) > 0):
                        # deferred block-0 staging (see above)
                        if 0 not in cbs0:
                            stage_sec(wtmp_pool, 0)
                        for bn in range(1, fb):
                            if bn not in cbs0:
                                stage_sec(wtmp_pool, bn)
                                break

                    w3list = []
                    r3s = []
                    if ch_carve and do_dve:
                        # K-side: the `carve` smallest-reach right-side terms
                        #   sum_j Bt_j relu(rsh_j - ym)^3,  ym = max(w, 0)
                        # R_j = Relu(rsh_j - ym) (ACT); R_j^3 per tune["kmeth"]
                        # ("sp": ACT Square + Pool mul, "pp": 2 Pool muls,
                        # "le": ACT Ln + Exp(scale=3)); rounded to fp32r; then
                        # one PSUM accumulation group on the tensor engine:
                        #   W3 = (-S0) x ones + sum_j diag(Bt_j) R3_j
                        # which the DVE chain's first term consumes as its
                        # in1 accumulator seed (DVE reads PSUM directly).
                        KC = tune["kchunk"]
                        kmeth = tune["kmeth"]
                        ym = ym_pool.tile([P, ctok], mybir.dt.float32, tag="ym")
                        if tune.get("ym_eng", "act") == "act":
                            nc.scalar.activation(
                                ym[:], wt, mybir.ActivationFunctionType.Relu,
                                bias=0.0, scale=1.0)
                        else:
                            nc.gpsimd.tensor_scalar_max(ym[:], wt, 0.0)
                        for j in range(carve - ch_carve, carve):
                            meth = kmeth[j % len(kmeth)]
                            R = r_pool.tile([P, ctok], mybir.dt.float32, tag="r")
                            nc.scalar.activation(
                                R[:], ym[:], mybir.ActivationFunctionType.Relu,
                                bias=aux[:, j:j + 1], scale=-1.0)
                            R3 = r2_pool.tile([P, ctok], mybir.dt.float32r,
                                              tag="r3")
                            if meth == "le":
                                L = t_pool.tile([P, ctok], mybir.dt.float32,
                                                tag="r2")
                                nc.scalar.activation(
                                    L[:], R[:],
                                    mybir.ActivationFunctionType.Ln)
                                nc.scalar.activation(
                                    R3[:], L[:],
                                    mybir.ActivationFunctionType.Exp,
                                    scale=3.0)
                            else:
                                R2 = t_pool.tile([P, ctok], mybir.dt.float32,
                                                 tag="r2")
                                if meth[0] == "s":
                                    nc.scalar.square(R2[:], R[:])
                                elif meth[0] == "d":
                                    nc.vector.tensor_mul(R2[:], R[:], R[:])
                                else:
                                    nc.gpsimd.tensor_mul(R2[:], R[:], R[:])
                                if meth[1] == "d":
                                    nc.vector.tensor_mul(R3[:], R2[:], R[:])
                                else:
                                    nc.gpsimd.tensor_mul(R3[:], R2[:], R[:])
                            r3s.append((j, R3))
                        # with w3t the carved terms merge in the output-
                        # transpose accumulation group (emit_out) instead of
                        # seeding the DVE chain — no W3 build here.
                        for so in ([] if tune.get("w3t", False)
                                   else range(0, tic, KC)):
                            sc = min(KC, tic - so)
                            cols = sc * P
                            o0 = so * P
                            W3 = psk_pool.tile([P, KC * P], mybir.dt.float32)
                            # matmuls are column-split at 512 (moving-dim and
                            # PSUM-bank limit); the DVE consumes the whole
                            # tile in one instruction
                            for h0 in range(0, cols, 512):
                                hc = min(512, cols - h0)
                                nc.tensor.matmul(
                                    W3[:, h0:h0 + hc],
                                    s0r[:, b * P:(b + 1) * P],
                                    ones[:, :hc], start=True, stop=False)
                                for ji, (j, R3) in enumerate(r3s):
                                    cw = (b * carve + j) * P
                                    nc.tensor.matmul(
                                        W3[:, h0:h0 + hc], wdr[:, cw:cw + P],
                                        R3[:, o0 + h0:o0 + h0 + hc],
                                        start=False,
                                        stop=(ji == len(r3s) - 1))
                            w3list.append((o0, cols, W3))

                    flush_out()
                    if do_dve:
                        acc_a = acc_pool.tile([P, ctok], mybir.dt.float32, tag="acc")
                        acc_b = acc_pool.tile([P, ctok], mybir.dt.float32, tag="acc")
                        cur, nxt = acc_a, acc_b
                        nR = 5 - ch_carve
                        if ch_carve and w3list:
                            # left term k=0 doubles as the W3+(-S0) merge:
                            # acc = W3 + A0 relu(min(w,0)-lsh0)^3
                            for o0, cols, W3 in w3list:
                                nc.vector._custom_dve(
                                    SPLINE_ACC_L, out=cur[:, o0:o0 + cols],
                                    in0=w[:, o0:o0 + cols], in1=W3[:, :cols],
                                    s0=col(0), s1=float(lsh[0]))
                        else:
                            nc.vector._custom_dve(SPLINE_INIT_L, out=cur[:],
                                                  in0=wt, in1=col(10),
                                                  s0=col(0), s1=float(lsh[0]))
                        for k in range(1, 5):
                            nc.vector._custom_dve(SPLINE_ACC_L, out=nxt[:],
                                                  in0=wt, in1=cur[:], s0=col(k),
                                                  s1=float(lsh[k]))
                            cur, nxt = nxt, cur
                        for k in range(nR):
                            nc.vector._custom_dve(SPLINE_ACC_R, out=nxt[:],
                                                  in0=wt, in1=cur[:],
                                                  s0=col(5 + k), s1=float(rsh[k]))
                            cur, nxt = nxt, cur
                    else:
                        cur = w

                    if do_trans:
                        dve_evac = (b == fb - 1 and
                                    c >= len(sizes) -
                                    tune.get("dve_tail_evac", 0))
                        w3t_c = ch_carve if tune.get("w3t", False) else 0
                        args_out = (b, cs, tic, cur, outst, dve_evac,
                                    tuple(r3s), w3t_c)
                        if tune.get("lag_out", False):
                            pending_out[0] = args_out
                        else:
                            emit_out(*args_out)
                    else:
                        cv = cur[:].rearrange("p (t f) -> p t f", f=P)
                        nc.sync.dma_start(
                            ys_v[b][:, cs:cs + tic, :], cv)

            flush_out()
            if carve:
                wtmp_pool_cm.__exit__(None, None, None)
            if psin_cm is not None:
                psin_cm.__exit__(None, None, None)
            if wps_cm is not None:
                wps_cm.__exit__(None, None, None)
    nc.compile()
    _PROGRAMS[key] = nc
    return nc


# --------------------------------------------------------------------------
# entry point
# --------------------------------------------------------------------------

_EXECUTORS = {}


def _get_executor(nc, chain=1):
    """Jitted 8-core SPMD executable for `nc`, cached so repeat kernel()
    calls don't re-trace/re-compile."""
    key = (id(nc), chain)
    if key in _EXECUTORS:
        return _EXECUTORS[key]
    import jax
    from jax.sharding import Mesh, PartitionSpec, NamedSharding
    from jax.experimental.shard_map import shard_map
    import concourse.bass2jax as b2j
    import concourse.mybir as _mb

    b2j.install_neuronx_cc_hook()
    partition_name = (nc.partition_id_tensor.name
                      if nc.partition_id_tensor else None)
    in_names, out_names, out_avals = [], [], []
    for alloc in nc.m.functions[0].allocations:
        if not isinstance(alloc, _mb.MemoryLocationSet):
            continue
        name = alloc.memorylocations[0].name
        if alloc.kind == "ExternalInput":
            if name != partition_name:
                in_names.append(name)
        elif alloc.kind == "ExternalOutput":
            out_names.append(name)
            out_avals.append(jax.core.ShapedArray(
                tuple(alloc.tensor_shape), _mb.dt.np(alloc.dtype)))
    n_params = len(in_names)
    all_names = list(in_names) + list(out_names)
    if partition_name is not None:
        all_names = all_names + [partition_name]

    def _body(*args):
        operands = list(args)
        if partition_name is not None:
            operands.append(b2j.partition_id_tensor())
        outs = b2j._bass_exec_p.bind(
            *operands,
            out_avals=tuple(out_avals),
            in_names=tuple(all_names),
            out_names=tuple(out_names),
            lowering_input_output_aliases=(),
            sim_require_finite=True,
            sim_require_nnan=True,
            nc=nc,
        )
        return tuple(outs)

    devices = jax.devices()[:N_CORES]
    mesh = Mesh(np.asarray(devices), ("core",))
    spec = PartitionSpec("core")
    fn = jax.jit(shard_map(_body, mesh=mesh,
                           in_specs=(spec,) * (n_params + len(out_names)),
                           out_specs=(spec,) * len(out_names),
                           check_rep=False),
                 keep_unused=True)
    sharding = NamedSharding(mesh, spec)
    dev_zeros = [jax.device_put(
        np.zeros((N_CORES * a.shape[0], *a.shape[1:]), a.dtype), sharding)
        for a in out_avals]
    ex = dict(fn=fn, in_names=in_names, out_names=out_names,
              out_avals=out_avals, sharding=sharding, zeros=dev_zeros)
    _EXECUTORS[key] = ex
    return ex


def kernel(x, knots, coeffs, scaler):
    x = np.ascontiguousarray(np.asarray(x, dtype=np.float32))
    Bsz, Ssz, F = x.shape
    A, Bt, lsh, rsh, S0, h, center = _build_tables(knots, coeffs, scaler)
    tab = _pack_tab(A.astype(np.float32), Bt.astype(np.float32),
                    S0.astype(np.float32), F)

    x2 = x.reshape(-1, F)
    if center != 0.0:
        x2 = x2 - np.float32(center)
    N = x2.shape[0]
    assert N % (N_CORES * P) == 0
    tok = N // N_CORES

    nc = build_program(tok, F, lsh, rsh)
    ex = _get_executor(nc)
    carve = DEFAULT_TUNE["carve"]
    per_in = {"xs": np.ascontiguousarray(x2),
              "tab": np.concatenate([tab] * N_CORES, axis=0),
              "ident": np.concatenate([np.eye(P, dtype=np.float32)] * N_CORES,
                                      axis=0)}
    if carve:
        wd = _pack_diag(Bt.astype(np.float32), rsh, carve, F)
        aux = _pack_aux(rsh, carve)
        per_in["wd"] = np.concatenate([wd] * N_CORES, axis=0)
        per_in["aux"] = np.concatenate([aux] * N_CORES, axis=0)
        kc = DEFAULT_TUNE["kchunk"]
        s0row = np.concatenate([(-S0).astype(np.float32).reshape(1, F),
                                np.ones((1, kc * P), np.float32)], axis=1)
        per_in["s0t"] = np.concatenate([s0row] * N_CORES, axis=0)
    per_in = {k: v for k, v in per_in.items() if k in ex["in_names"]}
    import jax
    args = [jax.device_put(per_in[n], ex["sharding"]) for n in ex["in_names"]]
    args += ex["zeros"]
    out = ex["fn"](*args)
    ys = np.asarray(out[ex["out_names"].index("ys")])
    return ys.reshape(Bsz, Ssz, F).astype(np.float32, copy=False)


def timing_run(x, knots, coeffs, scaler, iters=20, tune=None):
    """Time steady-state device execution with device-resident inputs.

    Returns (min_per_call_seconds, out_array)."""
    import time
    import jax

    x = np.ascontiguousarray(np.asarray(x, dtype=np.float32))
    Bsz, Ssz, F = x.shape
    A, Bt, lsh, rsh, S0, h, center = _build_tables(knots, coeffs, scaler)
    tab = _pack_tab(A.astype(np.float32), Bt.astype(np.float32),
                    S0.astype(np.float32), F)
    x2 = x.reshape(-1, F)
    if center != 0.0:
        x2 = x2 - np.float32(center)
    tok = x2.shape[0] // N_CORES
    nc = build_program(tok, F, lsh, rsh, tune=tune)
    ex = _get_executor(nc)

    per_in = {"xs": x2, "tab": np.concatenate([tab] * N_CORES, axis=0),
              "ident": np.concatenate([np.eye(P, dtype=np.float32)] * N_CORES,
                                      axis=0)}
    carve = {**DEFAULT_TUNE, **(tune or {})}["carve"]
    if carve:
        per_in["wd"] = np.concatenate(
            [_pack_diag(Bt.astype(np.float32), rsh, carve, F)] * N_CORES, 0)
        per_in["aux"] = np.concatenate([_pack_aux(rsh, carve)] * N_CORES, 0)
        kc = {**DEFAULT_TUNE, **(tune or {})}["kchunk"]
        s0row = np.concatenate([(-S0).astype(np.float32).reshape(1, F),
                                np.ones((1, kc * P), np.float32)], axis=1)
        per_in["s0t"] = np.concatenate([s0row] * N_CORES, axis=0)
    per_in = {k: v for k, v in per_in.items() if k in ex["in_names"]}
    dev_in = [jax.device_put(per_in[n], ex["sharding"]) for n in ex["in_names"]]
    dev_zero = ex["zeros"]

    fn = ex["fn"]
    out = fn(*dev_in, *dev_zero)
    jax.block_until_ready(out)
    if os.environ.get("SPLINE_ASYNC_TIMING", "1") == "1":
        # async-pipelined: launch all iters, block once; amortizes the
        # axon RPC round-trip which otherwise dominates and is noisy
        t0 = time.time()
        for _ in range(iters):
            out = fn(*dev_in, *dev_zero)
        jax.block_until_ready(out)
        dt = (time.time() - t0) / iters
    else:
        times = []
        for _ in range(iters):
            t0 = time.time()
            out = fn(*dev_in, *dev_zero)
            jax.block_until_ready(out)
            times.append(time.time() - t0)
        dt = min(times)
    ys = np.asarray(out[ex["out_names"].index("ys")])
    res = ys.reshape(N_CORES * tok, F).reshape(Bsz, Ssz, F)
    return dt, res



# revision 44
# speedup vs baseline: 1.0061x; 1.0061x over previous
"""Trainium2 Bass kernel for nn_BSplineActivation.

out[n, f] = sum_j basis_j(x[n, f]) * coeffs[f, j] * scaler[f]
with a cubic B-spline basis on a uniform shared knot grid.

Math: with xc = x - center (center = grid midpoint) and h the knot step,
the spline for feature f is evaluated exactly as

    S_f(xc) = SL_f(min(xc, 0)) + SR_f(max(xc, 0)) - S_f(0)
    SL_f(y) = sum_{k=0..4} A_fk  * relu(y - lsh_k)^3    lsh_k = (k-5) h < 0
    SR_f(y) = sum_{k=0..4} Bt_fk * relu(rsh_k - y)^3    rsh_k = (5-k) h > 0

(two-sided truncated-power representation; the min/max clamps make each
side exact on its half and constant on the other, and x outside the
extended grid lands exactly on 0).  Each term is one fused custom DVE op
    acc' = acc + a * relu(min(w,0) - q)^3
so the whole activation is 10 vector-engine instructions per tile.

Layout: features on partitions (per-feature coefficients become
per-partition scalars), tokens on the free dim.  x tiles are transposed
in via the tensor engine (exact movement mode) and transposed back out;
the scalar engine evacuates PSUM.  Data-parallel over 8 NeuronCores on
the flattened token dim.

Performance (per core, 2048x768 shard): ~144us cost-model / ~110-140us
measured via repeat-slope; vector-engine bound at its floor (10 fused
2-source fp32 passes are port-capped at 1 elem/cycle/lane; accumulation
requires 2 sources; any segment decomposition of this 13-dof spline with
vanishing ends needs >= 10 truncated terms).  Input DMA, both transpose
directions, and PSUM evacuation overlap to within ~3us of the DVE-only
ablation.  Uneven ramp/tail chunking (first block (1,3,12) token-tiles,
last (12,3,1)) starts the DVE chain after one transposed tile and
shrinks the output tail.

Engine-offload was explored in depth (tune["carve"]>0 keeps the working
implementation): the K smallest-reach right-side terms are moved off the
DVE as Bt_k*relu2(rsh_k-ym)*relu(rsh_k-ym) with ACT computing
Relu/Square, cubes multiplied on Pool or DVE, per-feature coefficients
applied by fp32r diagonal matmuls accumulating in PSUM on the otherwise
idle PE (1 cycle/row at moving dim >= 256; measured fp32r precision
~2e-4 relative, fine for the 2e-2 tolerance), S0 injected by a rank-1
(-S0)^T x ones matmul, and the PSUM tile consumed directly as the in1
accumulator seed of the DVE chain's first term (DVE reads PSUM, so no
evacuation).  Hard-won constraints: Pool-engine tensor_scalar is a slow
software Q7 path (~10us/2048 cols measured; ym therefore computes on ACT
as Relu(w)), ACT saturates on PSUM-evac + Relu + Square, every exact
decomposition needs >= 3 non-DVE passes per carved term (no relu^3
table function; PSUM accumulate-on-write is PE-only; DMA/Pool cannot
read PSUM; stt MACs are DVE-only), which caps the useful carve at 2.

Current default (w3t, carve=3): the carved terms no longer seed the DVE
chain through a PSUM W3 tile.  Instead each output-transpose PSUM slice
becomes an accumulation group:

    ps2[tok, f] = T(acc_dve) + sum_j (stationary=r3_j-token-tile) @
                                     (moving=diag(Bt_j) block)

i.e. the per-feature-weighted carved cubes land TRANSPOSED directly on
top of the output transpose (ordinary matmuls accumulate over the
is_transpose write; start=True on the transpose, stop=True on the last
diag matmul; skip_group_check needed).  This removes the W3->first-DVE-op
dependency entirely, so the DVE chain (INIT_L + 4 ACC_L + 2 ACC_R = 7
passes, S0 via the INIT op's C3 slot) never waits on the ACT/Pool
K-side, and the -S0 rank-1 seed matmul is gone.  Validated on HW:
rel err 1.268e-2 (fp32r rounding of three carved terms, k=2 with
R^3<=11.4 dominates; tolerance 2e-2), model 121.6us (c2blocks=(5,):
120.9) vs 129.1us for the best carve=2 merge design and 144.5us for the
pure 10-pass DVE floor.  DVE busy 105us, ACT 102us, PE 77us, Pool 68us —
DVE and ACT are now co-saturated; further carve (4+) dies on ACT
(2 passes/term is irreducible: Relu + Square; ln/exp cubes are 3
passes; Pool squares blow the measured-2x-slow Q7 budget).

Measurement: axon-tunnel HW slopes vary +-50% run to run; the
TimelineSim model deltas are the stable signal (prior session validated
model==HW slope).  Numerics are deterministic across runs.

Explored and rejected this session (model numbers): psum_w — PE
transposes write w straight to PSUM and DVE/ACT read it there, killing
the ACT input-evacuation pass (the 8x2KB PSUM caps the w ring at 2-3
chunks and the pipeline serializes: 149-193us; DVE PSUM access is also
+65ns/instr).  evac_in="dma"/Pool (cannot read PSUM).  16-bit DVE
(custom-op rows have no 2x/4x perf modes — InstCustomDveAnt reports
none; TSP fp32 SBUF does run 2x_2p but >=3 standard ops never beat one
fused pass; fp16 cubes fail the error budget anyway).  Multi-term
fused ops (8-stage v3 pipeline; one term needs 7).  Custom ACT pwp
table for the shared cardinal B-spline basis (act tables are a closed
enum keyed to compiled bkt/ctrl binaries).  tokens-on-partitions (PE
contracts over partitions, so per-feature scaling then has no cheap
engine).  tsplit=1 (+21us: coarser pipeline), out-DMA on the ACT hwdge
queue (+11us: ACT SEQ dispatch blocks the saturated ACT engine),
carve_pat interleavings (monotone between the pure configs).
"""

import os
import numpy as np

# The kernel executes through the axon PJRT backend; make sure a
# JAX_PLATFORMS=cpu pin (common for reference-only environments) does not
# hide the NeuronCore devices.  Must run before jax is first imported.
_jp = os.environ.get("JAX_PLATFORMS")
if _jp is not None and "axon" not in _jp:
    os.environ["JAX_PLATFORMS"] = "axon,cpu"

import concourse.bacc as bacc
import concourse.mybir as mybir
import concourse.tile as tile
from concourse import masks
from concourse.dve_spec import (
    Spec, Src0, Src1, C0, C1, C3, Zero, relu, sq, minn, maxx, lower,
    _spill_c3_to_src1, _has_src1,
)
from concourse.dve_ops import DveOp, OPS, _SUB_OPCODE_FOR_NAME, CUSTOM_DVE_SPECS
from concourse.dve_uop import DveOpSpec

N_CORES = 8
P = 128


# --------------------------------------------------------------------------
# custom DVE ops (registered once per process)
# --------------------------------------------------------------------------

def _register(name, spec):
    for op in OPS:
        if op.name == name:
            return op
    row = max(_SUB_OPCODE_FOR_NAME.values()) + 1
    assert row < 0x20, "no free custom-DVE opcode rows"
    _SUB_OPCODE_FOR_NAME[name] = row
    shas = {}
    for ver in ("v3", "v4"):
        try:
            uops = lower(spec, ver=ver)
            shas[ver] = DveOpSpec(
                name=name, opcode=row, uops=uops, rd1_en=_has_src1(spec)
            ).sha(ver)
        except Exception:
            if ver == "v3":
                raise
    op = DveOp(name, spec, subdim=False, uops_sha=shas)
    OPS.append(op)
    CUSTOM_DVE_SPECS[name] = spec
    return op


def _cube(r):
    return sq(r) * r


# acc + a * relu(min(w,0) - q)^3        (left-side truncated power term)
SPLINE_ACC_L = _register(
    "SPLINE_ACC_L_ANT",
    Spec(
        body=Src1 + C0 * _cube(relu(minn(Src0, Zero) - C1)),
        reference=lambda in0, in1, s0, s1, imm2: (
            in1 + s0 * np.maximum(np.minimum(in0.astype(np.float32), 0) - s1, 0) ** 3
        ).astype(np.float32),
    ),
)

# acc + a * relu(q - max(w,0))^3        (right-side term)
SPLINE_ACC_R = _register(
    "SPLINE_ACC_R_ANT",
    Spec(
        body=Src1 + C0 * _cube(relu(C1 - maxx(Src0, Zero))),
        reference=lambda in0, in1, s0, s1, imm2: (
            in1 + s0 * np.maximum(s1 - np.maximum(in0.astype(np.float32), 0), 0) ** 3
        ).astype(np.float32),
    ),
)

# a * relu(min(w,0) - q)^3 + c          (chain seed; c = -S(0) rides C3->Src1)
SPLINE_INIT_L = _register(
    "SPLINE_INIT_L_ANT",
    Spec(
        body=_spill_c3_to_src1(C0 * _cube(relu(minn(Src0, Zero) - C1)) + C3),
        reference=lambda in0, in1, s0, s1, imm2: (
            s0 * np.maximum(np.minimum(in0.astype(np.float32), 0) - s1, 0) ** 3 + in1
        ).astype(np.float32),
    ),
)


# --------------------------------------------------------------------------
# host-side table construction (exact, float64)
# --------------------------------------------------------------------------

def _build_tables(knots, coeffs, scaler):
    knots = np.asarray(knots, np.float64)
    coeffs = np.asarray(coeffs, np.float64)
    scaler = np.asarray(scaler, np.float64)
    F, G = knots.shape
    h = (knots[:, -1] - knots[:, 0]) / (G - 1)
    assert np.allclose(np.diff(knots, axis=1), h[:, None], rtol=0, atol=1e-5), \
        "kernel assumes uniform knots per feature"
    assert np.allclose(h, h[0], rtol=0, atol=1e-9), "kernel assumes shared knot step"
    h = float(h[0])
    center = (knots[:, 0] + knots[:, -1]) / 2
    assert np.allclose(center, center[0], atol=1e-9)
    center = float(center[0])

    c = coeffs * scaler[:, None]                       # (F, nb)
    nb = c.shape[1]
    w4 = np.array([1.0, -4.0, 6.0, -4.0, 1.0]) / 6.0

    dU = np.zeros((F, nb + 4))
    for j in range(nb):
        dU[:, j:j + 5] += c[:, j:j + 1] * w4[None, :]
    A = dU[:, :5] / h ** 3
    lsh = (np.arange(5) - 5.0) * h

    crev = c[:, ::-1]
    dUT = np.zeros((F, nb + 4))
    for j in range(nb):
        dUT[:, j:j + 5] += crev[:, j:j + 1] * w4[None, :]
    Bt = dUT[:, :5] / h ** 3
    rsh = (5.0 - np.arange(5)) * h

    S0 = (A * np.maximum(-lsh, 0.0)[None, :] ** 3).sum(1)
    return A, Bt, lsh, rsh, S0, h, center


NCOL = 12  # 5 left + 5 right + S0neg + pad


def _pack_tab(A, Bt, S0, F):
    fb = F // P
    tab = np.zeros((P, fb * NCOL), np.float32)
    for b in range(fb):
        sl = slice(b * P, (b + 1) * P)
        tab[:, b * NCOL + 0:b * NCOL + 5] = A[sl]
        tab[:, b * NCOL + 5:b * NCOL + 10] = Bt[sl]
        tab[:, b * NCOL + 10] = -S0[sl]
    return tab


def _pack_diag(Bt, rsh, carve, F):
    """Diagonal PE weight matrices diag(Bt_k) for the carved right-side
    terms (k = 5-carve .. 4), per feature block: (P, fb*carve*P)."""
    fb = F // P
    ks = list(range(5 - carve, 5))
    wd = np.zeros((P, fb * carve * P), np.float32)
    idx = np.arange(P)
    for b in range(fb):
        for j, k in enumerate(ks):
            col0 = (b * carve + j) * P
            wd[idx, col0 + idx] = Bt[b * P:(b + 1) * P, k]
    return wd


def _pack_aux(rsh, carve):
    aux = np.zeros((P, 8), np.float32)
    for j, k in enumerate(range(5 - carve, 5)):
        aux[:, j] = np.float32(rsh[k])
    return aux


# --------------------------------------------------------------------------
# bass program
# --------------------------------------------------------------------------

_PROGRAMS = {}


DEFAULT_TUNE = dict(xin=2, w=5, acc=5, outb=2, psin=2, psout=2, tsplit=2,
                    repeat=1, phase="full", evac=4, first_split=2,
                    last_split=2, first_sizes=(1, 2, 5, 8),
                    last_sizes=(8, 5, 2, 1),
                    # w3t: carved terms merge inside the output-transpose
                    # PSUM accumulation group (stationary = r3 token-tile,
                    # moving = diag weight block), so the DVE chain never
                    # waits on the ACT/Pool K-side.
                    w3t=True, lag_out=True, lag_depth=2, lazy_wd=True,
                    wd_defer=0, dve_head_evac=0, c2blocks=(5,),
                    # psum_w: PE transposes write the feature-major x tile
                    # directly into PSUM and every consumer (DVE chain, ACT
                    # ym) reads it there — the ACT PSUM->SBUF input
                    # evacuation pass disappears entirely.  Requires chunk
                    # sizes <= 8 token-tiles (PSUM bank budget: w 2x2 banks,
                    # psout 2, psk 2).
                    psum_w=False, wbufs=2,
                    # K-side carve (opt-in, default off): move `carve`
                    # smallest-reach right-side truncated-power terms off the
                    # DVE onto ACT (Relu/Square) + PE (fp32r diag-matmul
                    # chains accumulating in PSUM, consumed as the DVE
                    # chain's in1 seed).  Measured on HW: no win over the
                    # pure-DVE floor (Pool engine tensor ops are far slower
                    # than modeled; ACT saturates on evac + relu + square),
                    # so the default stays carve=0.
                    carve=3, kchunk=4, psk=1, rbuf=4, r2buf=8, ymbuf=2,
                    tbuf=3, kmeth=("sp", "sp", "sp"), ym_eng="act")


def build_program(tok, F, lsh, rsh, tune=None):
    """One-core program: xs (tok, F) f32 -> ys (tok, F) f32."""
    tune = {**DEFAULT_TUNE, **(tune or {})}
    key = (tok, F, tuple(lsh), tuple(rsh), tuple(sorted(tune.items())))
    if key in _PROGRAMS:
        return _PROGRAMS[key]

    fb = F // P
    ti = tok // P
    tsplit = tune["tsplit"]
    assert ti % tsplit == 0
    tic = ti // tsplit           # token-tiles per chunk
    ctok = tok // tsplit         # tokens per chunk
    psum_w = tune["psum_w"]
    if psum_w:
        assert tune["phase"] == "full"
        # largest chunk must fit the fixed [P, 8*P] PSUM w tile
        for key_ in ("first_sizes", "last_sizes"):
            if tune.get(key_):
                assert max(tune[key_]) <= 8, (key_, tune[key_])
        assert tic <= 8

    nc = bacc.Bacc("TRN2", target_bir_lowering=False, debug=False,
                   enable_asserts=False)
    xs = nc.dram_tensor("xs", (tok, F), mybir.dt.float32, kind="ExternalInput").ap()
    tabd = nc.dram_tensor("tab", (P, fb * NCOL), mybir.dt.float32,
                          kind="ExternalInput").ap()
    identd = (nc.dram_tensor("ident", (P, P), mybir.dt.float32,
                             kind="ExternalInput").ap()
              if tune.get("dma_ident", False) else None)
    carve = tune["carve"]
    if carve:
        wdd = nc.dram_tensor("wd", (P, fb * carve * P), mybir.dt.float32,
                             kind="ExternalInput").ap()
        auxd = nc.dram_tensor("aux", (P, 8), mybir.dt.float32,
                              kind="ExternalInput").ap()
        s0td = nc.dram_tensor("s0t", (1, fb * P + tune["kchunk"] * P),
                              mybir.dt.float32, kind="ExternalInput").ap()
    ys = nc.dram_tensor("ys", (tok, F), mybir.dt.float32, kind="ExternalOutput").ap()

    xs_v = xs.rearrange("(t p) (b f) -> b p t f", p=P, f=P)
    ys_v = ys.rearrange("(t p) (b f) -> b p t f", p=P, f=P)

    with tile.TileContext(nc) as tc:
        with (
            tc.tile_pool(name="consts", bufs=1) as consts,
            tc.tile_pool(name="xin_pool", bufs=tune["xin"]) as xin_pool,
            tc.tile_pool(name="w_pool", bufs=tune["w"]) as w_pool,
            tc.tile_pool(name="acc_pool", bufs=tune["acc"]) as acc_pool,
            tc.tile_pool(name="out_pool", bufs=tune["outb"]) as out_pool,
            tc.tile_pool(name="psout", bufs=tune["psout"], space="PSUM") as psout_pool,
            tc.tile_pool(name="psk", bufs=tune["psk"], space="PSUM") as psk_pool,
            tc.tile_pool(name="r_pool", bufs=tune["rbuf"]) as r_pool,
            tc.tile_pool(name="r2_pool", bufs=tune["r2buf"]) as r2_pool,
            tc.tile_pool(name="ym_pool", bufs=tune["ymbuf"]) as ym_pool,
            tc.tile_pool(name="t_pool", bufs=tune["tbuf"]) as t_pool,
        ):
            psin_cm = (None if psum_w else
                       tc.tile_pool(name="psin", bufs=tune["psin"], space="PSUM"))
            psin_pool = psin_cm.__enter__() if psin_cm is not None else None
            wps_cm = (tc.tile_pool(name="wps", bufs=tune["wbufs"], space="PSUM")
                      if psum_w else None)
            wps_pool = wps_cm.__enter__() if wps_cm is not None else None
            # first ramp chunk's input DMA ahead of every constant transfer:
            # the first DVE op waits on (DMA -> transpose -> evac) and the
            # SP queue is FIFO, so anything queued before this chunk delays
            # the whole pipeline head
            early_xin = None
            if tune.get("early_x", False) and tune["phase"] != "notrans":
                fs0 = tune.get("first_sizes")
                tic0 = (fs0[0] if fs0 and sum(fs0) == ti
                        else ti // (tune.get("first_split") or tune["tsplit"]))
                early_xin = xin_pool.tile([P, ti, P], mybir.dt.float32)
                nc.sync.dma_start(early_xin[:, 0:tic0, :],
                                  xs_v[0][:, 0:tic0, :])
            cq = (nc.scalar if tune.get("const_q", "sp") == "act"
                  else nc.sync)
            identity = consts.tile([P, P], mybir.dt.float32)
            if tune.get("dma_ident", False):
                cq.dma_start(identity[:], identd[:])
            else:
                masks.make_identity(nc, identity[:])
            tab = consts.tile([P, fb * NCOL], mybir.dt.float32)
            cq.dma_start(tab[:], tabd[:])
            if carve:
                wdr = consts.tile([P, fb * carve * P], mybir.dt.float32r)
                aux = consts.tile([P, 8], mybir.dt.float32)
                nko = fb * P + tune["kchunk"] * P
                s0f = consts.tile([1, nko], mybir.dt.float32)
                s0r = consts.tile([1, nko], mybir.dt.float32r)
                cq.dma_start(aux[:], auxd[:])
                cq.dma_start(s0f[:], s0td[:])
                nc.gpsimd.tensor_copy(s0r[:], s0f[:])
                ones = s0r[:, fb * P:]

            staged_secs = set()

            def stage_sec(wtmp_pool, sec):
                # stage + producer-round one per-block slice of the diag
                # weights to fp32r.  Emitted one block ahead of its consumer
                # so the Pool-engine converts never burst-collide with the
                # per-chunk cube multiplies (a 10us Pool burst here stalled
                # the W3 seed chain for a whole block).
                if sec in staged_secs or sec >= fb * tune["repeat"]:
                    return
                staged_secs.add(sec)
                sec = sec % fb
                c0 = sec * carve * P
                wt = wtmp_pool.tile([P, carve * P], mybir.dt.float32,
                                    tag="wt")
                dq = nc.scalar if tune.get("wd_q", "sp") == "act" else nc.sync
                dq.dma_start(wt[:], wdd[:, c0:c0 + carve * P])
                nc.gpsimd.tensor_copy(wdr[:, c0:c0 + carve * P], wt[:])

            def stage_wdr(wtmp_pool):
                for sec in range(fb):
                    stage_sec(wtmp_pool, sec)

            staged = [False]
            wtmp_pool_cm = tc.tile_pool(name="wtmp", bufs=2)
            wtmp_pool = wtmp_pool_cm.__enter__() if carve else None
            # lag_out: defer each chunk's output stage (PE out-transpose, ACT
            # PSUM evacuation, output DMA) until after the NEXT chunk's
            # K-side has been emitted.  The ACT queue is FIFO per engine, so
            # without this the next chunk's ym/Relu/Square — which gate the
            # next DVE chain via the W3 seed — wait behind the previous
            # chunk's output evacuation.
            pending_out = []

            def emit_out(b, cs, tic, cur, outst, dve_evac, r3s=(),
                         ch_carve=0):
                E = tune["evac"]
                fine_out = tune.get("fine_out", True)
                oq = (nc.scalar if (tune.get("out_q", "sp") == "act" or
                                    b in tune.get("out_q_blocks", ()))
                      else nc.sync)
                for t0 in range(0, tic, E):
                    ne = min(E, tic - t0)
                    ps2 = psout_pool.tile([P, E * P], mybir.dt.float32)
                    for e in range(ne):
                        tg0 = t0 + e
                        if ch_carve:
                            # w3t: the carved terms' per-feature-weighted
                            # cubes accumulate into the same PSUM slice the
                            # output transpose writes: ps2 = T(acc_dve)
                            # + sum_j (diag(Bt_j) r3_j)^T.  Stationary =
                            # r3 token-tile, moving = the diag block, so
                            # the product lands token-major directly and
                            # the DVE chain never waits on the K side.
                            nc.tensor.matmul(
                                ps2[:, e * P:(e + 1) * P],
                                cur[:, tg0 * P:(tg0 + 1) * P],
                                identity[:], is_transpose=True,
                                start=True, stop=False,
                                skip_group_check=True)
                            for ji, (j, R3) in enumerate(r3s):
                                cw = (b * carve + j) * P
                                nc.tensor.matmul(
                                    ps2[:, e * P:(e + 1) * P],
                                    R3[:, tg0 * P:(tg0 + 1) * P],
                                    wdr[:, cw:cw + P],
                                    start=False, stop=(ji == len(r3s) - 1),
                                    skip_group_check=True)
                        else:
                            nc.tensor.transpose(ps2[:, e * P:(e + 1) * P],
                                                cur[:, tg0 * P:(tg0 + 1) * P],
                                                identity[:])
                    if dve_evac:
                        # tail chunks evacuate on the DVE, which is idle by
                        # then, so the drain never queues behind ACT work
                        nc.vector.tensor_copy(
                            outst[:, cs + t0:cs + t0 + ne, :],
                            ps2[:, :ne * P])
                    else:
                        nc.scalar.copy(
                            outst[:, cs + t0:cs + t0 + ne, :],
                            ps2[:, :ne * P])
                    if fine_out:
                        oq.dma_start(
                            ys_v[b][:, cs + t0:cs + t0 + ne, :],
                            outst[:, cs + t0:cs + t0 + ne, :])
                if not fine_out:
                    oq.dma_start(ys_v[b][:, cs:cs + tic, :],
                                      outst[:, cs:cs + tic, :])

            def flush_out(force=True):
                depth = 0 if force else tune.get("lag_depth", 1) - 1
                while len(pending_out) > depth:
                    emit_out(*pending_out.pop(0))
            for b in range(fb * tune["repeat"]):
                b = b % fb
                tsplit = tune["tsplit"]
                sizes = None
                if (b == 0 and tune.get("first_sizes")
                        and sum(tune["first_sizes"]) == ti):
                    sizes = list(tune["first_sizes"])
                elif (b == fb - 1 and tune.get("last_sizes")
                        and sum(tune["last_sizes"]) == ti):
                    sizes = list(tune["last_sizes"])
                elif b == 0 and tune["first_split"]:
                    tsplit = tune["first_split"]
                elif b == fb - 1 and tune["last_split"]:
                    tsplit = tune["last_split"]
                if sizes is None:
                    assert ti % tsplit == 0
                    sizes = [ti // tsplit] * tsplit
                assert sum(sizes) == ti
                starts = [sum(sizes[:i]) for i in range(len(sizes))]
                def col(j, b=b):
                    return tab[:, b * NCOL + j:b * NCOL + j + 1]

                phase = tune["phase"]
                do_trans = phase in ("full", "nodve")
                do_dve = phase in ("full", "notrans")
                cbs0 = tune.get("cblk_skip", (0,))
                if cbs0 is True:
                    cbs0 = (0, fb - 1)
                elif cbs0 is False:
                    cbs0 = ()
                if carve and tune.get("lazy_wd", False):
                    if b != 0 or tune.get("wd_defer", 1) == 0:
                        # block 0's staging is deferred until after its
                        # first chunk's input DMA (emitted below) so the wd
                        # transfers never delay the first transposes
                        if b not in cbs0:
                            stage_sec(wtmp_pool, b)
                        for bn in range(b + 1, b + fb):
                            if bn % fb not in cbs0:
                                stage_sec(wtmp_pool, bn % fb)
                                break
                elif carve and not staged[0] and b not in cbs0:
                    stage_wdr(wtmp_pool)
                    staged[0] = True

                outst = out_pool.tile([P, ti, P], mybir.dt.float32)
                if do_trans:
                    if b == 0 and early_xin is not None:
                        xin = early_xin
                    else:
                        xin = xin_pool.tile([P, ti, P], mybir.dt.float32)
                for c, (cs, tic) in enumerate(zip(starts, sizes)):
                    ctok = tic * P
                    # small ramp chunks and (optionally) the first/last
                    # feature blocks skip the carve (full DVE chain) so the
                    # K-side pipeline never gates the head/tail; block 0 may
                    # use its own larger threshold (cmin0) so only its final
                    # ramp chunk carves, after the K-side pipeline warmed up
                    # behind the earlier full-DVE chunks
                    cmin = (tune.get("cmin0", 4) if b == 0
                            else tune.get("cmin", 4))
                    ch_carve = carve if tic >= cmin else 0
                    cpat = tune.get("carve_pat")
                    if cpat and ch_carve:
                        # cycle the carve depth across chunks: balances the
                        # DVE chain length against the ACT-side K latency
                        ch_carve = min(ch_carve,
                                       cpat[(b * len(sizes) + c) % len(cpat)])
                    if b in tune.get("c2blocks", ()):
                        # blocks where the ACT queue is the limiter run a
                        # shallower carve
                        ch_carve = min(ch_carve, 2)
                    cbs = tune.get("cblk_skip", (0,))
                    if cbs is True:
                        cbs = (0, fb - 1)
                    elif cbs is False:
                        cbs = ()
                    if b in cbs:
                        ch_carve = 0
                    if psum_w:
                        # fixed-size PSUM tile (2 banks); chunk uses a prefix
                        w = wps_pool.tile([P, 8 * P], mybir.dt.float32,
                                          tag="w")
                        nc.sync.dma_start(xin[:, cs:cs + tic, :],
                                          xs_v[b][:, cs:cs + tic, :])
                        for e in range(tic):
                            nc.tensor.transpose(w[:, e * P:(e + 1) * P],
                                                xin[:, cs + e, :],
                                                identity[:])
                    else:
                        w = w_pool.tile([P, ctok], mybir.dt.float32, tag="w")
                    wt = w[:, :ctok]
                    if do_trans and not psum_w:
                        if not (b == 0 and c == 0 and early_xin is not None):
                            nc.sync.dma_start(xin[:, cs:cs + tic, :],
                                              xs_v[b][:, cs:cs + tic, :])
                        E = tune.get("evac_in_g") or tune["evac"]
                        for t0 in range(0, tic, E):
                            ne = min(E, tic - t0)
                            ps = psin_pool.tile([P, E * P], mybir.dt.float32)
                            for e in range(ne):
                                tg = cs + t0 + e
                                nc.tensor.transpose(ps[:, e * P:(e + 1) * P],
                                                    xin[:, tg, :], identity[:])
                            if ((b == 0 and
                                    c < tune.get("dve_head_evac", 1)) or
                                    b in tune.get("dve_evac_blocks", ())):
                                # head chunks evacuate on the DVE so the
                                # first spline op never waits on the ACT
                                # table load; late blocks can use the
                                # draining DVE to relieve the ACT backlog
                                nc.vector.tensor_copy(
                                    w[:, t0 * P:(t0 + ne) * P],
                                    ps[:, :ne * P])
                            elif tune.get("evac_in", "act") == "dma":
                                nc.sync.dma_start(w[:, t0 * P:(t0 + ne) * P],
                                                  ps[:, :ne * P])
                            else:
                                nc.scalar.copy(w[:, t0 * P:(t0 + ne) * P],
                                               ps[:, :ne * P])
                    elif not do_trans:
                        wv = w[:].rearrange("p (t f) -> p t f", f=P)
                        nc.sync.dma_start(
                            wv, xs_v[b][:, cs:cs + tic, :])
                    if (carve and tune.get("lazy_wd", False) and b == 0
                            and c == tune.get("wd_defer", 1) - 1
                            and tune.get("wd_defer", 1) > 0):
                        # deferred block-0 staging (see above)
                        if 0 not in cbs0:
                            stage_sec(wtmp_pool, 0)
                        for bn in range(1, fb):
                            if bn not in cbs0:
                                stage_sec(wtmp_pool, bn)
                                break

                    w3list = []
                    r3s = []
                    if ch_carve and do_dve:
                        # K-side: the `carve` smallest-reach right-side terms
                        #   sum_j Bt_j relu(rsh_j - ym)^3,  ym = max(w, 0)
                        # R_j = Relu(rsh_j - ym) (ACT); R_j^3 per tune["kmeth"]
                        # ("sp": ACT Square + Pool mul, "pp": 2 Pool muls,
                        # "le": ACT Ln + Exp(scale=3)); rounded to fp32r; then
                        # one PSUM accumulation group on the tensor engine:
                        #   W3 = (-S0) x ones + sum_j diag(Bt_j) R3_j
                        # which the DVE chain's first term consumes as its
                        # in1 accumulator seed (DVE reads PSUM directly).
                        KC = tune["kchunk"]
                        kmeth = tune["kmeth"]
                        ym = ym_pool.tile([P, ctok], mybir.dt.float32, tag="ym")
                        if tune.get("ym_eng", "act") == "act":
                            nc.scalar.activation(
                                ym[:], wt, mybir.ActivationFunctionType.Relu,
                                bias=0.0, scale=1.0)
                        else:
                            nc.gpsimd.tensor_scalar_max(ym[:], wt, 0.0)
                        for j in range(carve - ch_carve, carve):
                            meth = kmeth[j % len(kmeth)]
                            R = r_pool.tile([P, ctok], mybir.dt.float32, tag="r")
                            nc.scalar.activation(
                                R[:], ym[:], mybir.ActivationFunctionType.Relu,
                                bias=aux[:, j:j + 1], scale=-1.0)
                            R3 = r2_pool.tile([P, ctok], mybir.dt.float32r,
                                              tag="r3")
                            if meth == "le":
                                L = t_pool.tile([P, ctok], mybir.dt.float32,
                                                tag="r2")
                                nc.scalar.activation(
                                    L[:], R[:],
                                    mybir.ActivationFunctionType.Ln)
                                nc.scalar.activation(
                                    R3[:], L[:],
                                    mybir.ActivationFunctionType.Exp,
                                    scale=3.0)
                            else:
                                R2 = t_pool.tile([P, ctok], mybir.dt.float32,
                                                 tag="r2")
                                if meth[0] == "s":
                                    nc.scalar.square(R2[:], R[:])
                                elif meth[0] == "d":
                                    nc.vector.tensor_mul(R2[:], R[:], R[:])
                                else:
                                    nc.gpsimd.tensor_mul(R2[:], R[:], R[:])
                                if meth[1] == "d":
                                    nc.vector.tensor_mul(R3[:], R2[:], R[:])
                                else:
                                    nc.gpsimd.tensor_mul(R3[:], R2[:], R[:])
                            r3s.append((j, R3))
                        # with w3t the carved terms merge in the output-
                        # transpose accumulation group (emit_out) instead of
                        # seeding the DVE chain — no W3 build here.
                        for so in ([] if tune.get("w3t", False)
                                   else range(0, tic, KC)):
                            sc = min(KC, tic - so)
                            cols = sc * P
                            o0 = so * P
                            W3 = psk_pool.tile([P, KC * P], mybir.dt.float32)
                            # matmuls are column-split at 512 (moving-dim and
                            # PSUM-bank limit); the DVE consumes the whole
                            # tile in one instruction
                            for h0 in range(0, cols, 512):
                                hc = min(512, cols - h0)
                                nc.tensor.matmul(
                                    W3[:, h0:h0 + hc],
                                    s0r[:, b * P:(b + 1) * P],
                                    ones[:, :hc], start=True, stop=False)
                                for ji, (j, R3) in enumerate(r3s):
                                    cw = (b * carve + j) * P
                                    nc.tensor.matmul(
                                        W3[:, h0:h0 + hc], wdr[:, cw:cw + P],
                                        R3[:, o0 + h0:o0 + h0 + hc],
                                        start=False,
                                        stop=(ji == len(r3s) - 1))
                            w3list.append((o0, cols, W3))

                    flush_out(force=False)
                    if do_dve:
                        acc_a = acc_pool.tile([P, ctok], mybir.dt.float32, tag="acc")
                        acc_b = acc_pool.tile([P, ctok], mybir.dt.float32, tag="acc")
                        cur, nxt = acc_a, acc_b
                        nR = 5 - ch_carve
                        if ch_carve and w3list:
                            # left term k=0 doubles as the W3+(-S0) merge:
                            # acc = W3 + A0 relu(min(w,0)-lsh0)^3
                            for o0, cols, W3 in w3list:
                                nc.vector._custom_dve(
                                    SPLINE_ACC_L, out=cur[:, o0:o0 + cols],
                                    in0=w[:, o0:o0 + cols], in1=W3[:, :cols],
                                    s0=col(0), s1=float(lsh[0]))
                        else:
                            nc.vector._custom_dve(SPLINE_INIT_L, out=cur[:],
                                                  in0=wt, in1=col(10),
                                                  s0=col(0), s1=float(lsh[0]))
                        for k in range(1, 5):
                            nc.vector._custom_dve(SPLINE_ACC_L, out=nxt[:],
                                                  in0=wt, in1=cur[:], s0=col(k),
                                                  s1=float(lsh[k]))
                            cur, nxt = nxt, cur
                        for k in range(nR):
                            nc.vector._custom_dve(SPLINE_ACC_R, out=nxt[:],
                                                  in0=wt, in1=cur[:],
                                                  s0=col(5 + k), s1=float(rsh[k]))
                            cur, nxt = nxt, cur
                    else:
                        cur = w

                    if do_trans:
                        dve_evac = (b == fb - 1 and
                                    c >= len(sizes) -
                                    tune.get("dve_tail_evac", 0))
                        w3t_c = ch_carve if tune.get("w3t", False) else 0
                        args_out = (b, cs, tic, cur, outst, dve_evac,
                                    tuple(r3s), w3t_c)
                        if tune.get("lag_out", False):
                            pending_out.append(args_out)
                        else:
                            emit_out(*args_out)
                    else:
                        cv = cur[:].rearrange("p (t f) -> p t f", f=P)
                        nc.sync.dma_start(
                            ys_v[b][:, cs:cs + tic, :], cv)

            flush_out()
            if carve:
                wtmp_pool_cm.__exit__(None, None, None)
            if psin_cm is not None:
                psin_cm.__exit__(None, None, None)
            if wps_cm is not None:
                wps_cm.__exit__(None, None, None)
    nc.compile()
    _PROGRAMS[key] = nc
    return nc


# --------------------------------------------------------------------------
# entry point
# --------------------------------------------------------------------------

_EXECUTORS = {}


def _get_executor(nc, chain=1):
    """Jitted 8-core SPMD executable for `nc`, cached so repeat kernel()
    calls don't re-trace/re-compile."""
    key = (id(nc), chain)
    if key in _EXECUTORS:
        return _EXECUTORS[key]
    import jax
    from jax.sharding import Mesh, PartitionSpec, NamedSharding
    from jax.experimental.shard_map import shard_map
    import concourse.bass2jax as b2j
    import concourse.mybir as _mb

    b2j.install_neuronx_cc_hook()
    partition_name = (nc.partition_id_tensor.name
                      if nc.partition_id_tensor else None)
    in_names, out_names, out_avals = [], [], []
    for alloc in nc.m.functions[0].allocations:
        if not isinstance(alloc, _mb.MemoryLocationSet):
            continue
        name = alloc.memorylocations[0].name
        if alloc.kind == "ExternalInput":
            if name != partition_name:
                in_names.append(name)
        elif alloc.kind == "ExternalOutput":
            out_names.append(name)
            out_avals.append(jax.core.ShapedArray(
                tuple(alloc.tensor_shape), _mb.dt.np(alloc.dtype)))
    n_params = len(in_names)
    all_names = list(in_names) + list(out_names)
    if partition_name is not None:
        all_names = all_names + [partition_name]

    def _body(*args):
        operands = list(args)
        if partition_name is not None:
            operands.append(b2j.partition_id_tensor())
        outs = b2j._bass_exec_p.bind(
            *operands,
            out_avals=tuple(out_avals),
            in_names=tuple(all_names),
            out_names=tuple(out_names),
            lowering_input_output_aliases=(),
            sim_require_finite=True,
            sim_require_nnan=True,
            nc=nc,
        )
        return tuple(outs)

    devices = jax.devices()[:N_CORES]
    mesh = Mesh(np.asarray(devices), ("core",))
    spec = PartitionSpec("core")
    fn = jax.jit(shard_map(_body, mesh=mesh,
                           in_specs=(spec,) * (n_params + len(out_names)),
                           out_specs=(spec,) * len(out_names),
                           check_rep=False),
                 keep_unused=True)
    sharding = NamedSharding(mesh, spec)
    dev_zeros = [jax.device_put(
        np.zeros((N_CORES * a.shape[0], *a.shape[1:]), a.dtype), sharding)
        for a in out_avals]
    ex = dict(fn=fn, in_names=in_names, out_names=out_names,
              out_avals=out_avals, sharding=sharding, zeros=dev_zeros)
    _EXECUTORS[key] = ex
    return ex


def kernel(x, knots, coeffs, scaler):
    x = np.ascontiguousarray(np.asarray(x, dtype=np.float32))
    Bsz, Ssz, F = x.shape
    A, Bt, lsh, rsh, S0, h, center = _build_tables(knots, coeffs, scaler)
    tab = _pack_tab(A.astype(np.float32), Bt.astype(np.float32),
                    S0.astype(np.float32), F)

    x2 = x.reshape(-1, F)
    if center != 0.0:
        x2 = x2 - np.float32(center)
    N = x2.shape[0]
    assert N % (N_CORES * P) == 0
    tok = N // N_CORES

    nc = build_program(tok, F, lsh, rsh)
    ex = _get_executor(nc)
    carve = DEFAULT_TUNE["carve"]
    per_in = {"xs": np.ascontiguousarray(x2),
              "tab": np.concatenate([tab] * N_CORES, axis=0),
              "ident": np.concatenate([np.eye(P, dtype=np.float32)] * N_CORES,
                                      axis=0)}
    if carve:
        wd = _pack_diag(Bt.astype(np.float32), rsh, carve, F)
        aux = _pack_aux(rsh, carve)
        per_in["wd"] = np.concatenate([wd] * N_CORES, axis=0)
        per_in["aux"] = np.concatenate([aux] * N_CORES, axis=0)
        kc = DEFAULT_TUNE["kchunk"]
        s0row = np.concatenate([(-S0).astype(np.float32).reshape(1, F),
                                np.ones((1, kc * P), np.float32)], axis=1)
        per_in["s0t"] = np.concatenate([s0row] * N_CORES, axis=0)
    per_in = {k: v for k, v in per_in.items() if k in ex["in_names"]}
    import jax
    args = [jax.device_put(per_in[n], ex["sharding"]) for n in ex["in_names"]]
    args += ex["zeros"]
    out = ex["fn"](*args)
    ys = np.asarray(out[ex["out_names"].index("ys")])
    return ys.reshape(Bsz, Ssz, F).astype(np.float32, copy=False)


def timing_run(x, knots, coeffs, scaler, iters=20, tune=None):
    """Time steady-state device execution with device-resident inputs.

    Returns (min_per_call_seconds, out_array)."""
    import time
    import jax

    x = np.ascontiguousarray(np.asarray(x, dtype=np.float32))
    Bsz, Ssz, F = x.shape
    A, Bt, lsh, rsh, S0, h, center = _build_tables(knots, coeffs, scaler)
    tab = _pack_tab(A.astype(np.float32), Bt.astype(np.float32),
                    S0.astype(np.float32), F)
    x2 = x.reshape(-1, F)
    if center != 0.0:
        x2 = x2 - np.float32(center)
    tok = x2.shape[0] // N_CORES
    nc = build_program(tok, F, lsh, rsh, tune=tune)
    ex = _get_executor(nc)

    per_in = {"xs": x2, "tab": np.concatenate([tab] * N_CORES, axis=0),
              "ident": np.concatenate([np.eye(P, dtype=np.float32)] * N_CORES,
                                      axis=0)}
    carve = {**DEFAULT_TUNE, **(tune or {})}["carve"]
    if carve:
        per_in["wd"] = np.concatenate(
            [_pack_diag(Bt.astype(np.float32), rsh, carve, F)] * N_CORES, 0)
        per_in["aux"] = np.concatenate([_pack_aux(rsh, carve)] * N_CORES, 0)
        kc = {**DEFAULT_TUNE, **(tune or {})}["kchunk"]
        s0row = np.concatenate([(-S0).astype(np.float32).reshape(1, F),
                                np.ones((1, kc * P), np.float32)], axis=1)
        per_in["s0t"] = np.concatenate([s0row] * N_CORES, axis=0)
    per_in = {k: v for k, v in per_in.items() if k in ex["in_names"]}
    dev_in = [jax.device_put(per_in[n], ex["sharding"]) for n in ex["in_names"]]
    dev_zero = ex["zeros"]

    fn = ex["fn"]
    out = fn(*dev_in, *dev_zero)
    jax.block_until_ready(out)
    if os.environ.get("SPLINE_ASYNC_TIMING", "1") == "1":
        # async-pipelined: launch all iters, block once; amortizes the
        # axon RPC round-trip which otherwise dominates and is noisy
        t0 = time.time()
        for _ in range(iters):
            out = fn(*dev_in, *dev_zero)
        jax.block_until_ready(out)
        dt = (time.time() - t0) / iters
    else:
        times = []
        for _ in range(iters):
            t0 = time.time()
            out = fn(*dev_in, *dev_zero)
            jax.block_until_ready(out)
            times.append(time.time() - t0)
        dt = min(times)
    ys = np.asarray(out[ex["out_names"].index("ys")])
    res = ys.reshape(N_CORES * tok, F).reshape(Bsz, Ssz, F)
    return dt, res

